# revision 15
# baseline (speedup 1.0000x reference)
"""Trainium2 Bass kernel for the ARqGPSFull autoregressive wavefunction.

Math: out[b] = sum_{s,m} ctx[b,s,m] * I_{x[b,s]}[s,m]; in logs the masked
product is affine in x, so with centered x' = x-0.5 and the observed-state
selection folded into the diagonal (see _host_pack):
  T[b,s,m] = exp(Sre) * (cos(Sim) + i sin(Sim)),  S = x'^T D + C
Each core owns 8 of the 64 m-values -> 512 (s,m) columns.  D ships as a
bf16 hi+lo pair (two accumulating matmuls, 66+64 contraction rows) so S is
fp32-exact; the constants ride rows 64/65 of the hi block, split hi/lo.

v4 structure (vs the 20.0us baseline):
- ONE act table set for the whole kernel: pwp set 22 (exp_and_friends)
  holds exp AND sin2pi, killing the 1283ns mid-kernel table switch.
  sin2pi shares opcode 4 with sin; walrus knows the BIR name "Sin2pi"
  (to_json_bytes shim), inputs pre-scaled by 1/(2pi).  Probe: exact to
  6e-8 on |x| <= 0.5 including the edges.
- No range reduction: half-angle identities with a direct cosine.
    sq = sin(Sim/2)                       (|Sim/2| < pi, in domain)
    cq = cos(Sim/2) = sin2pi(Sim/(4pi) + 1/4)   (bias rides the ACT op)
    cos(Sim) = 1 - 2 sq^2,  sin(Sim) = 2 sq cq
  ACT is the only PSUM reader (cross-engine same-bank PSUM reads
  serialize), and Pool touches no 512-wide f32 tensors (it runs them
  2-3x slower than DVE and contends for SBUF).
- Product-sums collapse onto the common factor u = pe*sq:
    o1 = sum(2 u cq) = Tim      o2 = sum(2 u sq) = aE - Tre
  so the tail is 3 ACT ops (exp+2 sins) + 3 DVE ops (u, scrR, scrI) +
  a [B,1] Pool subtract + the DVE block-transpose.
- 4 matmuls of N=512 (re-hi 66, re-lo 64, im-hi 66, im-lo 64); exp waits
  only the two re matmuls.
- 3 input DMAs grouped by arrival need and spread across the three
  DMA-capable engines so no single sequencer serializes the issues:
  [XT|RH] -> SP, [RL] -> ACT, [IMH|IML] -> Pool; all hoisted pre-barrier
  post-compile together with the act-table load; the preamble Pool DMA
  drain is deleted.  Output: one DMA on ACT + one on SP (same-queue
  equal-shape DMAs get mis-merged by the compiler).
"""

import sys

for _p in ("/opt/trn_rl_repo", "/root/.axon_site/_ro/trn_rl_repo"):
    if _p not in sys.path:
        sys.path.append(_p)

import math
import numpy as np
import ml_dtypes

N_CORES = 8
B = 128        # batch
L = 64         # n_sites
M = 64         # GPS support dim
NM = M // N_CORES   # m-values per core
NBLK = L * NM  # 512 (s,m) columns per core
PI = math.pi

_BF16 = ml_dtypes.bfloat16

_built = None

# rhs column layout: DMA-a = [XT|RH], DMA-b = [RL], DMA-c = [IMH|IML]
C_XT = 0
C_RH = B
C_RL = C_RH + NBLK
C_IMH = C_RL + NBLK
C_IML = C_IMH + NBLK
C_TOT = C_IML + NBLK


def _build():
    import concourse.bacc as bacc
    import concourse.mybir as mybir
    from concourse import tile

    f32 = mybir.dt.float32
    bf16 = mybir.dt.bfloat16
    AF = mybir.ActivationFunctionType
    ALU = mybir.AluOpType

    nc = bacc.Bacc()
    rhs_d = nc.dram_tensor("rhs", [66, C_TOT], bf16, kind="ExternalInput")
    out_d = nc.dram_tensor("out", [2, 4, 32], f32, kind="ExternalOutput")

    with tile.TileContext(nc) as tc:
        with (
            tc.tile_pool(name="pc", bufs=1) as pc,
            tc.tile_pool(name="psum", bufs=1, space="PSUM") as psum,
        ):
            rhs = pc.tile([66, C_TOT], bf16, tag="rhs")
            # three input DMAs: the critical [XT|RH] chunk first on SP
            # (fast HW-queue issue, lands first), [RL] second on SP,
            # [IMH|IML] on Pool -- the configuration whose chunk-0 landed
            # earliest across all measured variants.
            nc.sync.dma_start(rhs[:, C_XT:C_RL], rhs_d[:, C_XT:C_RL])
            nc.sync.dma_start(rhs[:, C_RL:C_IMH], rhs_d[:, C_RL:C_IMH])
            nc.gpsimd.dma_start(rhs[:, C_IMH:C_TOT], rhs_d[:, C_IMH:C_TOT])
            xt66 = rhs[:, C_XT:C_XT + B]
            xt64 = rhs[0:64, C_XT:C_XT + B]

            o = pc.tile([B, 32], f32, tag="o")
            nc.gpsimd.memset(o[:], 0.0)
            quarter = pc.tile([B, 1], f32, tag="quarter")
            nc.gpsimd.memset(quarter[:], 0.25)

            Sr = psum.tile([B, NBLK], f32, tag="Sr")
            Si = psum.tile([B, NBLK], f32, tag="Si")
            pe = pc.tile([B, NBLK], f32, tag="pe")
            nc.tensor.matmul(Sr[:], xt66, rhs[:, C_RH:C_RH + NBLK],
                             start=True, stop=False)
            nc.tensor.matmul(Sr[:], xt64, rhs[0:64, C_RL:C_RL + NBLK],
                             start=False, stop=True)
            # pe = exp(Sre), aE = sum(pe) free from the ACT accumulator
            nc.scalar.activation(pe[:], Sr[:], AF.Exp, accum_out=o[:, 0:1])
            nc.tensor.matmul(Si[:], xt66, rhs[:, C_IMH:C_IMH + NBLK],
                             start=True, stop=False)
            nc.tensor.matmul(Si[:], xt64, rhs[0:64, C_IML:C_IML + NBLK],
                             start=False, stop=True)

            # sq = sin(Sim/2), cq = cos(Sim/2): ACT only, no range reduction
            sq = pc.tile([B, NBLK], f32, tag="sq")
            cq = pc.tile([B, NBLK], f32, tag="cq")
            nc.scalar.activation(sq[:], Si[:], AF.Sin, scale=0.25 / PI)
            nc.scalar.activation(cq[:], Si[:], AF.Sin, scale=0.25 / PI,
                                 bias=quarter[:])

            # u = pe*sq; o1 = sum(2 u cq) = Tim; o2 = sum(2 u sq) = aE - Tre
            u = pc.tile([B, NBLK], f32, tag="u")
            nc.vector.tensor_mul(u[:], pe[:], sq[:])
            scrR = pc.tile([B, NBLK], f32, tag="scrR")
            nc.vector.scalar_tensor_tensor(
                scrR[:], u[:], 2.0, sq[:], op0=ALU.mult, op1=ALU.mult,
                accum_out=o[:, 2:3])
            scrI = pc.tile([B, NBLK], f32, tag="scrI")
            nc.vector.scalar_tensor_tensor(
                scrI[:], u[:], 2.0, cq[:], op0=ALU.mult, op1=ALU.mult,
                accum_out=o[:, 1:2])
            # oRe = aE - sum(2 u sq), [B,1] on Pool
            nc.gpsimd.tensor_sub(o[:, 2:3], o[:, 0:1], o[:, 2:3])

            # block-transpose so the output DMA rows are contiguous:
            # tr[32k+c, p] = o[32k+p, c]
            tr = pc.tile([B, 32], f32, tag="tr")
            nc.vector.transpose(tr[:], o[:])
            nc.gpsimd.dma_start(out_d[0], tr[2:99:32, :])
            nc.sync.dma_start(out_d[1], tr[1:98:32, :])

    nc.compile()

    import os
    mybir_ET = mybir.EngineType

    # --- single act table: set 22 (exp + sin2pi), delete other loads ----
    loads = [(b, ins) for b in nc.main_func.blocks
             for ins in b.instructions
             if type(ins).__name__ == "InstLoadActFuncSet"]
    assert loads, "no act table load found"
    loads[0][1].act_func_set_id = 22
    for b, ins in loads[1:]:
        si = ins.sync_info
        assert si is None or (not si.on_wait and not si.on_update)
        b.instructions.remove(ins)

    # Sin -> Sin2pi in the serialized BIR (walrus-native name)
    _orig_json = nc.to_json_bytes
    nc.to_json_bytes = (
        lambda: _orig_json().replace(b'"func":"Sin"', b'"func":"Sin2pi"'))

    # --- pin the PE stream to re-hi, re-lo, im-hi, im-lo ----------------
    # tile schedules im-hi before re-lo (its DMA lands first), which makes
    # exp wait three matmuls instead of two.  Swap the (ldweights, matmul)
    # pairs back and relax exp's gate to PE>=2.
    b1s = nc.main_func.blocks[1]

    def _mm_waits(ins, name):
        si = ins.sync_info
        return si is not None and any(w.ant_name.startswith(name)
                                      for w in si.on_wait)

    pe_idx = [i for i, ins in enumerate(b1s.instructions)
              if ins.engine == mybir.EngineType.PE
              and type(ins).__name__ in ("InstLdweights", "InstMatmult")]
    pairs = [(pe_idx[k], pe_idx[k + 1]) for k in range(0, len(pe_idx), 2)]
    im_hi = next((p for p in pairs
                  if _mm_waits(b1s.instructions[p[1]], "DMASW0")), None)
    re_lo = next((p for p in pairs
                  if _mm_waits(b1s.instructions[p[1]], "DMAHW1")), None)
    if im_hi and re_lo and im_hi[0] < re_lo[0]:
        ins_list = b1s.instructions
        a_ldw, a_mm = ins_list[im_hi[0]], ins_list[im_hi[1]]
        b_ldw, b_mm = ins_list[re_lo[0]], ins_list[re_lo[1]]
        ins_list[im_hi[0]], ins_list[im_hi[1]] = b_ldw, b_mm
        ins_list[re_lo[0]], ins_list[re_lo[1]] = a_ldw, a_mm
        for ins in ins_list:
            if (ins.engine == mybir.EngineType.Activation
                    and type(ins).__name__ == "InstEventSemaphore"):
                si = ins.sync_info
                if si and any(w.ant_name.startswith("PE_")
                              and w.wait_value == 3 for w in si.on_wait):
                    for w in si.on_wait:
                        if w.ant_name.startswith("PE_"):
                            w.wait_value = 2
                    break

    if os.environ.get("NO_HOIST") == "1":
        return nc

    # --- hoist wait-free input DMAs + act table load into the preamble --
    b0, b1 = nc.main_func.blocks[0], nc.main_func.blocks[1]
    hoist = []
    for ins in list(b1.instructions):
        nm = type(ins).__name__
        if nm == "InstDMACopy" and ins.engine in (mybir_ET.Pool,
                                                  mybir_ET.SP,
                                                  mybir_ET.Activation):
            si = ins.sync_info
            if si is not None and si.on_wait:
                continue  # output DMAs wait on results
            hoist.append(ins)
            b1.instructions.remove(ins)
        elif nm == "InstLoadActFuncSet":
            si = ins.sync_info
            assert si is None or (not si.on_wait and not si.on_update)
            hoist.append(ins)
            b1.instructions.remove(ins)
    for ins in reversed(hoist):
        first = next((i for i, x in enumerate(b0.instructions)
                      if x.engine == ins.engine), len(b0.instructions))
        b0.instructions.insert(first, ins)
    if os.environ.get("KEEP_DRAIN") != "1":
        for ins in list(b0.instructions):
            if (type(ins).__name__ == "InstDrain"
                    and ins.engine == mybir_ET.Pool):
                b0.instructions.remove(ins)
    return nc


def _host_pack(inputs, params_context, inputs_param):
    x = np.asarray(inputs).astype(np.float64)          # (B, L) in {0,1}
    P = np.asarray(params_context)                     # (s, d, m, j) complex
    I = np.asarray(inputs_param)                       # (s, d, m) complex

    mask = (np.arange(L)[None, :] < np.maximum(np.arange(L), 1)[:, None])
    Lp = np.log(P)
    D = (Lp[:, 1] - Lp[:, 0]) * mask[:, None, :]       # (s, m, j)
    C = (Lp[:, 0] * mask[:, None, :]).sum(-1)          # (s, m)
    I0 = I[:, 0]
    I1 = I[:, 1]
    A0 = np.log(np.abs(I0))
    dA = np.log(np.abs(I1)) - A0
    wrap = lambda t: np.angle(np.exp(1j * t))
    ph0 = np.angle(I0)
    dPh = wrap(np.angle(I1) - ph0)
    eye = np.eye(L)[:, None, :]                        # (s, 1, j)
    Dre = D.real + eye * dA[:, :, None]                # (s, m, j)
    Dim = D.imag + eye * dPh[:, :, None]
    CA = C.real + A0 + 0.5 * Dre.sum(-1)               # x-centering shift
    PH = wrap(C.imag + ph0 + 0.5 * Dim.sum(-1))

    xt = np.concatenate([(x - 0.5).T, np.ones((2, B))], 0)  # (66, B)
    rhs_list = []
    for k in range(N_CORES):
        msl = slice(k * NM, (k + 1) * NM)
        full = np.zeros((66, C_TOT), np.float64)
        full[:, C_XT:C_XT + B] = xt
        for Dp, const, chi, clo in (
                (Dre, CA, C_RH, C_RL),
                (Dim, PH, C_IMH, C_IML)):
            Dc = Dp[:, msl, :].transpose(2, 0, 1).reshape(L, NBLK)  # (j, sm)
            Dhi = Dc.astype(_BF16).astype(np.float64)
            Dlo = Dc - Dhi
            cc = const[:, msl].reshape(NBLK)
            hi = cc.astype(_BF16).astype(np.float64)
            lo = cc - hi
            full[0:64, chi:chi + NBLK] = Dhi
            full[0:64, clo:clo + NBLK] = Dlo
            full[64, chi:chi + NBLK] = hi
            full[65, chi:chi + NBLK] = lo
        rhs_list.append(full.astype(_BF16))
    return rhs_list


def kernel(inputs, params_context, inputs_param):
    global _built
    from concourse.bass_utils import run_bass_kernel_spmd

    if _built is None:
        _built = _build()
    nc = _built

    rhs_list = _host_pack(inputs, params_context, inputs_param)
    in_maps = [{"rhs": rhs_list[k]} for k in range(N_CORES)]
    res = run_bass_kernel_spmd(nc, in_maps, list(range(N_CORES)))

    re = np.zeros(B, np.float64)
    im = np.zeros(B, np.float64)
    for k in range(N_CORES):
        q = np.asarray(res.results[k]["out"], np.float64)  # (2, 4, 32)
        re += q[0].reshape(B)
        im += q[1].reshape(B)   # o1 = +Tim in this formulation
    return (re + 1j * np.angle(np.exp(1j * im))).astype(np.complex128)


# revision 16
# speedup vs baseline: 1.1661x; 1.1661x over previous
"""Trainium2 Bass kernel for the ARqGPSFull autoregressive wavefunction.

Math: out[b] = sum_{s,m} ctx[b,s,m] * I_{x[b,s]}[s,m]; in logs the masked
product is affine in x, so with centered x' = x-0.5 and the observed-state
selection folded into the diagonal (see _host_pack):
  T[b,s,m] = exp(Sre) * (cos(Sim) + i sin(Sim)),  S = x'^T D + C
Each core owns 8 of the 64 m-values -> 512 (s,m) columns.  D ships as a
bf16 hi+lo pair (two accumulating matmuls, 66+64 contraction rows) so S is
fp32-exact; the constants ride rows 64/65 of the hi block, split hi/lo.

v4 structure (vs the 20.0us baseline):
- ONE act table set for the whole kernel: pwp set 22 (exp_and_friends)
  holds exp AND sin2pi, killing the 1283ns mid-kernel table switch.
  sin2pi shares opcode 4 with sin; walrus knows the BIR name "Sin2pi"
  (to_json_bytes shim), inputs pre-scaled by 1/(2pi).  Probe: exact to
  6e-8 on |x| <= 0.5 including the edges.
- No range reduction: half-angle identities with a direct cosine.
    sq = sin(Sim/2)                       (|Sim/2| < pi, in domain)
    cq = cos(Sim/2) = sin2pi(Sim/(4pi) + 1/4)   (bias rides the ACT op)
    cos(Sim) = 1 - 2 sq^2,  sin(Sim) = 2 sq cq
  ACT is the only PSUM reader (cross-engine same-bank PSUM reads
  serialize), and Pool touches no 512-wide f32 tensors (it runs them
  2-3x slower than DVE and contends for SBUF).
- Product-sums collapse onto the common factor u = pe*sq:
    o1 = sum(2 u cq) = Tim      o2 = sum(2 u sq) = aE - Tre
  so the tail is 3 ACT ops (exp+2 sins) + 3 DVE ops (u, scrR, scrI) +
  a [B,1] Pool subtract + the DVE block-transpose.
- 4 matmuls of N=512 (re-hi 66, re-lo 64, im-hi 66, im-lo 64); exp waits
  only the two re matmuls.
- 3 input DMAs grouped by arrival need and spread across the three
  DMA-capable engines so no single sequencer serializes the issues:
  [XT|RH] -> SP, [RL] -> ACT, [IMH|IML] -> Pool; all hoisted pre-barrier
  post-compile together with the act-table load; the preamble Pool DMA
  drain is deleted.  Output: one DMA on ACT + one on SP (same-queue
  equal-shape DMAs get mis-merged by the compiler).
"""

import sys

for _p in ("/opt/trn_rl_repo", "/root/.axon_site/_ro/trn_rl_repo"):
    if _p not in sys.path:
        sys.path.append(_p)

import math
import numpy as np
import ml_dtypes

N_CORES = 8
B = 128        # batch
L = 64         # n_sites
M = 64         # GPS support dim
NM = M // N_CORES   # m-values per core
NBLK = L * NM  # 512 (s,m) columns per core
PI = math.pi

_BF16 = ml_dtypes.bfloat16

_built = None

# rhs column layout: DMA-a = [XT|RH], DMA-b = [RL], DMA-c = [IMH|IML]
C_XT = 0
C_RH = B
C_RL = C_RH + NBLK
C_IMH = C_RL + NBLK
C_IML = C_IMH + NBLK
C_TOT = C_IML + NBLK


def _build():
    import concourse.bacc as bacc
    import concourse.mybir as mybir
    from concourse import tile

    f32 = mybir.dt.float32
    bf16 = mybir.dt.bfloat16
    AF = mybir.ActivationFunctionType
    ALU = mybir.AluOpType

    nc = bacc.Bacc()
    rhs_d = nc.dram_tensor("rhs", [66, C_TOT], bf16, kind="ExternalInput")
    out_d = nc.dram_tensor("out", [2, 4, 32], f32, kind="ExternalOutput")

    with tile.TileContext(nc) as tc:
        with (
            tc.tile_pool(name="pc", bufs=1) as pc,
            tc.tile_pool(name="psum", bufs=1, space="PSUM") as psum,
        ):
            rhs = pc.tile([66, C_TOT], bf16, tag="rhs")
            # three input DMAs: the critical [XT|RH] chunk first on SP
            # (fast HW-queue issue, lands first), [RL] second on SP,
            # [IMH|IML] on Pool -- the configuration whose chunk-0 landed
            # earliest across all measured variants.
            nc.sync.dma_start(rhs[:, C_XT:C_RL], rhs_d[:, C_XT:C_RL])
            nc.sync.dma_start(rhs[:, C_RL:C_IMH], rhs_d[:, C_RL:C_IMH])
            nc.gpsimd.dma_start(rhs[:, C_IMH:C_TOT], rhs_d[:, C_IMH:C_TOT])
            xt66 = rhs[:, C_XT:C_XT + B]
            xt64 = rhs[0:64, C_XT:C_XT + B]

            o = pc.tile([B, 32], f32, tag="o")
            nc.gpsimd.memset(o[:], 0.0)
            quarter = pc.tile([B, 1], f32, tag="quarter")
            nc.gpsimd.memset(quarter[:], 0.25)

            Sr = psum.tile([B, NBLK], f32, tag="Sr")
            Si = psum.tile([B, NBLK], f32, tag="Si")
            pe = pc.tile([B, NBLK], f32, tag="pe")
            nc.tensor.matmul(Sr[:], xt66, rhs[:, C_RH:C_RH + NBLK],
                             start=True, stop=False)
            nc.tensor.matmul(Sr[:], xt64, rhs[0:64, C_RL:C_RL + NBLK],
                             start=False, stop=True)
            # pe = exp(Sre), aE = sum(pe) free from the ACT accumulator
            nc.scalar.activation(pe[:], Sr[:], AF.Exp, accum_out=o[:, 0:1])
            nc.tensor.matmul(Si[:], xt66, rhs[:, C_IMH:C_IMH + NBLK],
                             start=True, stop=False)
            nc.tensor.matmul(Si[:], xt64, rhs[0:64, C_IML:C_IML + NBLK],
                             start=False, stop=True)

            # sq = sin(Sim/2), cq = cos(Sim/2): ACT only, no range reduction
            sq = pc.tile([B, NBLK], f32, tag="sq")
            cq = pc.tile([B, NBLK], f32, tag="cq")
            nc.scalar.activation(sq[:], Si[:], AF.Sin, scale=0.25 / PI)
            nc.scalar.activation(cq[:], Si[:], AF.Sin, scale=0.25 / PI,
                                 bias=quarter[:])

            # u = pe*sq; o1 = sum(2 u cq) = Tim; o2 = sum(2 u sq) = aE - Tre
            u = pc.tile([B, NBLK], f32, tag="u")
            nc.vector.tensor_mul(u[:], pe[:], sq[:])
            scrR = pc.tile([B, NBLK], f32, tag="scrR")
            nc.vector.scalar_tensor_tensor(
                scrR[:], u[:], 2.0, sq[:], op0=ALU.mult, op1=ALU.mult,
                accum_out=o[:, 2:3])
            scrI = pc.tile([B, NBLK], f32, tag="scrI")
            nc.vector.scalar_tensor_tensor(
                scrI[:], u[:], 2.0, cq[:], op0=ALU.mult, op1=ALU.mult,
                accum_out=o[:, 1:2])
            # oRe = aE - sum(2 u sq), [B,1] on Pool
            nc.gpsimd.tensor_sub(o[:, 2:3], o[:, 0:1], o[:, 2:3])

            # block-transpose so the output DMA rows are contiguous:
            # tr[32k+c, p] = o[32k+p, c]
            tr = pc.tile([B, 32], f32, tag="tr")
            nc.vector.transpose(tr[:], o[:])
            nc.gpsimd.dma_start(out_d[0], tr[2:99:32, :])
            nc.sync.dma_start(out_d[1], tr[1:98:32, :])

    nc.compile()

    import os
    mybir_ET = mybir.EngineType

    # --- single act table: set 22 (exp + sin2pi), delete other loads ----
    loads = [(b, ins) for b in nc.main_func.blocks
             for ins in b.instructions
             if type(ins).__name__ == "InstLoadActFuncSet"]
    assert loads, "no act table load found"
    loads[0][1].act_func_set_id = 22
    for b, ins in loads[1:]:
        si = ins.sync_info
        assert si is None or (not si.on_wait and not si.on_update)
        b.instructions.remove(ins)

    # Sin -> Sin2pi in the serialized BIR (walrus-native name)
    _orig_json = nc.to_json_bytes
    nc.to_json_bytes = (
        lambda: _orig_json().replace(b'"func":"Sin"', b'"func":"Sin2pi"'))



    if os.environ.get("NO_HOIST") == "1":
        return nc

    # --- hoist wait-free input DMAs + act table load into the preamble --
    b0, b1 = nc.main_func.blocks[0], nc.main_func.blocks[1]
    hoist = []
    for ins in list(b1.instructions):
        nm = type(ins).__name__
        if nm == "InstDMACopy" and ins.engine in (mybir_ET.Pool,
                                                  mybir_ET.SP,
                                                  mybir_ET.Activation):
            si = ins.sync_info
            if si is not None and si.on_wait:
                continue  # output DMAs wait on results
            hoist.append(ins)
            b1.instructions.remove(ins)
        elif nm == "InstLoadActFuncSet":
            si = ins.sync_info
            assert si is None or (not si.on_wait and not si.on_update)
            hoist.append(ins)
            b1.instructions.remove(ins)
    for ins in reversed(hoist):
        first = next((i for i, x in enumerate(b0.instructions)
                      if x.engine == ins.engine), len(b0.instructions))
        b0.instructions.insert(first, ins)
    if os.environ.get("KEEP_DRAIN") != "1":
        for ins in list(b0.instructions):
            if (type(ins).__name__ == "InstDrain"
                    and ins.engine == mybir_ET.Pool):
                b0.instructions.remove(ins)
    return nc


def _host_pack(inputs, params_context, inputs_param):
    x = np.asarray(inputs).astype(np.float64)          # (B, L) in {0,1}
    P = np.asarray(params_context)                     # (s, d, m, j) complex
    I = np.asarray(inputs_param)                       # (s, d, m) complex

    mask = (np.arange(L)[None, :] < np.maximum(np.arange(L), 1)[:, None])
    Lp = np.log(P)
    D = (Lp[:, 1] - Lp[:, 0]) * mask[:, None, :]       # (s, m, j)
    C = (Lp[:, 0] * mask[:, None, :]).sum(-1)          # (s, m)
    I0 = I[:, 0]
    I1 = I[:, 1]
    A0 = np.log(np.abs(I0))
    dA = np.log(np.abs(I1)) - A0
    wrap = lambda t: np.angle(np.exp(1j * t))
    ph0 = np.angle(I0)
    dPh = wrap(np.angle(I1) - ph0)
    eye = np.eye(L)[:, None, :]                        # (s, 1, j)
    Dre = D.real + eye * dA[:, :, None]                # (s, m, j)
    Dim = D.imag + eye * dPh[:, :, None]
    CA = C.real + A0 + 0.5 * Dre.sum(-1)               # x-centering shift
    PH = wrap(C.imag + ph0 + 0.5 * Dim.sum(-1))

    xt = np.concatenate([(x - 0.5).T, np.ones((2, B))], 0)  # (66, B)
    rhs_list = []
    for k in range(N_CORES):
        msl = slice(k * NM, (k + 1) * NM)
        full = np.zeros((66, C_TOT), np.float64)
        full[:, C_XT:C_XT + B] = xt
        for Dp, const, chi, clo in (
                (Dre, CA, C_RH, C_RL),
                (Dim, PH, C_IMH, C_IML)):
            Dc = Dp[:, msl, :].transpose(2, 0, 1).reshape(L, NBLK)  # (j, sm)
            Dhi = Dc.astype(_BF16).astype(np.float64)
            Dlo = Dc - Dhi
            cc = const[:, msl].reshape(NBLK)
            hi = cc.astype(_BF16).astype(np.float64)
            lo = cc - hi
            full[0:64, chi:chi + NBLK] = Dhi
            full[0:64, clo:clo + NBLK] = Dlo
            full[64, chi:chi + NBLK] = hi
            full[65, chi:chi + NBLK] = lo
        rhs_list.append(full.astype(_BF16))
    return rhs_list


def kernel(inputs, params_context, inputs_param):
    global _built
    from concourse.bass_utils import run_bass_kernel_spmd

    if _built is None:
        _built = _build()
    nc = _built

    rhs_list = _host_pack(inputs, params_context, inputs_param)
    in_maps = [{"rhs": rhs_list[k]} for k in range(N_CORES)]
    res = run_bass_kernel_spmd(nc, in_maps, list(range(N_CORES)))

    re = np.zeros(B, np.float64)
    im = np.zeros(B, np.float64)
    for k in range(N_CORES):
        q = np.asarray(res.results[k]["out"], np.float64)  # (2, 4, 32)
        re += q[0].reshape(B)
        im += q[1].reshape(B)   # o1 = +Tim in this formulation
    return (re + 1j * np.angle(np.exp(1j * im))).astype(np.complex128)
